# revision 4
# baseline (speedup 1.0000x reference)
"""AttentionalPropagation (SuperGlue-style GNN message passing) on 8 TRN2 NeuronCores.

Sharding: pure data parallel over the batch dim (B=8 -> one batch element per core).
Per-core computation (x, src are (256, 2048) slices; all matmuls in bf16, f32 accum):

  Q = WqS @ x + bq          (256, 2048)   stacked-head layout, c = h*64+dh
  K = WkS @ s + bk          (256, 2048)
  VT = s^T @ WvS^T + bv     (2048, 256)   keys on partitions (transposed layout)
  per head h: S^T[m,n] = K_h[:,m] . Q_h[:,n]  -> exp(S^T/8)  (no max-subtraction;
      scores are O(1) so exp is safe in fp32/bf16)
  msg_u[dh,n] = sum_m exp . VT[m, h*64+dh];  den[n] = sum_m exp   (augmented-V matmul)
  msg = msg_u / den  -> message = WmP @ msg + bm
  h1 = W1 @ [x; message]    (b1 dropped: InstanceNorm cancels any per-channel bias)
  hn = relu((h1 - mean) * rstd)  over n
  out = W2 @ hn + b2
"""

import os
import sys

for _p in ("/opt/trn_rl_repo",):
    if _p not in sys.path:
        sys.path.insert(0, _p)

import numpy as np
import ml_dtypes

import concourse.bass as bass
import concourse.mybir as mybir
from concourse import bacc
from concourse import library_config
from concourse.bass import ts
from concourse.tile import TileContext
from concourse.bass_utils import run_bass_kernel_spmd

F32 = mybir.dt.float32
BF16 = mybir.dt.bfloat16

B, D, N, M, H, DH = 8, 256, 2048, 2048, 4, 64
EPS = 1e-5
NCH = 4  # n-chunks of 512
CHUNK = 512


def _build():
    nc = bacc.Bacc("TRN2", target_bir_lowering=False, debug=False, num_devices=8)

    x_d = nc.dram_tensor("x", [D, N], BF16, kind="ExternalInput").ap()
    s_d = nc.dram_tensor("src", [D, M], BF16, kind="ExternalInput").ap()
    wq_d = nc.dram_tensor("wqT", [D, D], BF16, kind="ExternalInput").ap()
    wk_d = nc.dram_tensor("wkT", [D, D], BF16, kind="ExternalInput").ap()
    wv_d = nc.dram_tensor("wvT", [D, D], BF16, kind="ExternalInput").ap()
    wm_d = nc.dram_tensor("wmT", [D, D], BF16, kind="ExternalInput").ap()
    w1_d = nc.dram_tensor("w1T", [2 * D, 2 * D], BF16, kind="ExternalInput").ap()
    w2_d = nc.dram_tensor("w2T", [2 * D, D], BF16, kind="ExternalInput").ap()
    bq_d = nc.dram_tensor("bq", [D, 1], F32, kind="ExternalInput").ap()
    bk_d = nc.dram_tensor("bk", [D, 1], F32, kind="ExternalInput").ap()
    bv_d = nc.dram_tensor("bv", [1, D], BF16, kind="ExternalInput").ap()
    bm_d = nc.dram_tensor("bm", [D, 1], F32, kind="ExternalInput").ap()
    b2_d = nc.dram_tensor("b2", [D, 1], F32, kind="ExternalInput").ap()
    out_d = nc.dram_tensor("out", [D, N], F32, kind="ExternalOutput").ap()

    with TileContext(nc) as tc:
        nc.gpsimd.load_library(library_config.attn)
        with (
            tc.tile_pool(name="const", bufs=1) as const,
            tc.tile_pool(name="data", bufs=1) as data,
            tc.tile_pool(name="exps", bufs=4) as exps,
            tc.tile_pool(name="small", bufs=2) as small,
            tc.tile_pool(name="msgn", bufs=3) as msgn,
            tc.tile_pool(name="ps_sc", bufs=2, space="PSUM") as ps_sc,
            tc.tile_pool(name="ps_msg", bufs=2, space="PSUM") as ps_msg,
            tc.tile_pool(name="ps_gen", bufs=2, space="PSUM") as ps_gen,
        ):
            # ---- constants / weights ----
            wq_sb = [const.tile([128, D], BF16, name=f"wq{k}") for k in range(2)]
            wk_sb = [const.tile([128, D], BF16, name=f"wk{k}") for k in range(2)]
            wv_sb = [const.tile([128, D], BF16, name=f"wv{k}") for k in range(2)]
            wm_sb = [const.tile([128, D], BF16, name=f"wm{k}") for k in range(2)]
            w1_sb = [const.tile([128, 2 * D], BF16, name=f"w1{k}") for k in range(4)]
            w2_sb = [const.tile([128, D], BF16, name=f"w2{k}") for k in range(4)]
            for k in range(2):
                nc.sync.dma_start(out=wq_sb[k][:], in_=wq_d[ts(k, 128), :])
                nc.sync.dma_start(out=wk_sb[k][:], in_=wk_d[ts(k, 128), :])
                nc.sync.dma_start(out=wv_sb[k][:], in_=wv_d[ts(k, 128), :])
                nc.sync.dma_start(out=wm_sb[k][:], in_=wm_d[ts(k, 128), :])
            for k in range(4):
                nc.sync.dma_start(out=w1_sb[k][:], in_=w1_d[ts(k, 128), :])
                nc.sync.dma_start(out=w2_sb[k][:], in_=w2_d[ts(k, 128), :])
            bq_sb = [const.tile([128, 1], F32, name=f"bq{c}") for c in range(2)]
            bk_sb = [const.tile([128, 1], F32, name=f"bk{c}") for c in range(2)]
            bm_sb = [const.tile([128, 1], F32, name=f"bm{c}") for c in range(2)]
            b2_sb = [const.tile([128, 1], F32, name=f"b2{c}") for c in range(2)]
            for c in range(2):
                nc.sync.dma_start(out=bq_sb[c][:], in_=bq_d[ts(c, 128), :])
                nc.sync.dma_start(out=bk_sb[c][:], in_=bk_d[ts(c, 128), :])
                nc.sync.dma_start(out=bm_sb[c][:], in_=bm_d[ts(c, 128), :])
                nc.sync.dma_start(out=b2_sb[c][:], in_=b2_d[ts(c, 128), :])
            # bv broadcast to all partitions (DMA partition-stride-0 from DRAM)
            bv_bc = const.tile([128, D], BF16, name="bvbc")
            bv_src = bass.AP(
                tensor=bv_d.tensor, offset=bv_d.offset, ap=[[0, 128]] + bv_d.ap[1:]
            )
            nc.sync.dma_start(out=bv_bc[:], in_=bv_src)
            eps_sb = const.tile([128, 1], F32, name="eps")
            nc.vector.memset(eps_sb[:], EPS)

            # ---- inputs ----
            x_sb = [data.tile([128, N], BF16, name=f"x{c}") for c in range(2)]
            s_sb = [data.tile([128, M], BF16, name=f"s{c}") for c in range(2)]
            for c in range(2):
                nc.sync.dma_start(out=x_sb[c][:], in_=x_d[ts(c, 128), :])
                nc.sync.dma_start(out=s_sb[c][:], in_=s_d[ts(c, 128), :])

            # ---- phase 1: QKV projections ----
            q_sb = [data.tile([128, N], BF16, name=f"q{c}") for c in range(2)]
            k_sb = [data.tile([128, M], BF16, name=f"k{c}") for c in range(2)]
            for c in range(2):  # output channel tile
                for j in range(NCH):
                    qp = ps_gen.tile([128, CHUNK], F32, name="qkps", tag="gps")
                    for k in range(2):
                        nc.tensor.matmul(
                            qp[:],
                            wq_sb[k][:, ts(c, 128)],
                            x_sb[k][:, ts(j, CHUNK)],
                            start=(k == 0),
                            stop=(k == 1),
                        )
                    nc.vector.tensor_scalar_add(
                        q_sb[c][:, ts(j, CHUNK)], qp[:], bq_sb[c][:]
                    )
                    kp = ps_gen.tile([128, CHUNK], F32, name="qkps", tag="gps")
                    for k in range(2):
                        nc.tensor.matmul(
                            kp[:],
                            wk_sb[k][:, ts(c, 128)],
                            s_sb[k][:, ts(j, CHUNK)],
                            start=(k == 0),
                            stop=(k == 1),
                        )
                    nc.vector.tensor_scalar_add(
                        k_sb[c][:, ts(j, CHUNK)], kp[:], bk_sb[c][:]
                    )
            # V^T: (m, c) layout, 65-wide per-head blocks with a ones column
            vT_sb = [data.tile([128, H, DH + 1], BF16, name=f"vT{t}") for t in range(16)]
            for t in range(16):
                vp = ps_gen.tile([128, D], F32, name="vps", tag="gps")
                for k in range(2):
                    nc.tensor.matmul(
                        vp[:],
                        s_sb[k][:, ts(t, 128)],
                        wv_sb[k][:],
                        start=(k == 0),
                        stop=(k == 1),
                    )
                nc.vector.tensor_add(
                    vT_sb[t][:, :, 0:DH],
                    vp[:].rearrange("p (h d) -> p h d", h=H),
                    bv_bc[:].rearrange("p (h d) -> p h d", h=H),
                )
                nc.vector.memset(vT_sb[t][:, :, DH : DH + 1], 1.0)

            # ---- phase 2: attention + Wm + W1, per n-chunk ----
            h1_sb = [data.tile([128, N], F32, name=f"h1{o}") for o in range(4)]
            stats_sb = [data.tile([128, NCH, 6], F32, name=f"st{o}") for o in range(4)]
            for j in range(NCH):
                eS = {}
                for p in range(2):  # head pair
                    for h2 in range(2):  # head within pair
                        eS[2 * p + h2] = exps.tile(
                            [128, 16, CHUNK], BF16, name="expS", tag="expS"
                        )
                    for s in range(8):  # m-super tile: 2 m-tiles
                        scp = [
                            ps_sc.tile([128, 2, CHUNK], F32, name="scps")
                            for _ in range(2)
                        ]
                        for jj in range(2):
                            mt = 2 * s + jj
                            for h2 in range(2):
                                nc.tensor.matmul(
                                    scp[h2][:, jj, :],
                                    k_sb[p][ts(h2, DH), ts(mt, 128)],
                                    q_sb[p][ts(h2, DH), ts(j, CHUNK)],
                                    start=True,
                                    stop=True,
                                )
                        for h2 in range(2):
                            nc.scalar.activation(
                                eS[2 * p + h2][:, 2 * s : 2 * s + 2, :],
                                scp[h2][:],
                                mybir.ActivationFunctionType.Exp,
                                scale=1.0 / 8.0,
                            )
                # msg per head (augmented-V: row 64 = denominator)
                mn_sb = [msgn.tile([128, CHUNK], BF16, name=f"mn{p}") for p in range(2)]
                for h in range(4):
                    p, h2 = h // 2, h % 2
                    mp = ps_msg.tile([DH + 1, CHUNK], F32, name="msgps")
                    for mt in range(16):
                        nc.tensor.matmul(
                            mp[:],
                            vT_sb[mt][:, h, :],
                            eS[h][:, mt, :],
                            start=(mt == 0),
                            stop=(mt == 15),
                        )
                    den = small.tile([1, CHUNK], F32, name="den", tag="den")
                    nc.vector.tensor_copy(den[:], mp[DH : DH + 1, :])
                    rden = small.tile([1, CHUNK], F32, name="rden", tag="rden")
                    nc.vector.reciprocal_approx_fast(rden[:], den[:])
                    rbc = small.tile([DH, CHUNK], F32, name="rbc", tag="rbc")
                    nc.gpsimd.partition_broadcast(rbc[:], rden[:])
                    nc.vector.tensor_mul(
                        mn_sb[p][ts(h2, DH), :], mp[0:DH, :], rbc[:]
                    )
                # message = WmP @ msg + bm
                msg_sb = [msgn.tile([128, CHUNK], BF16, name=f"ms{e}") for e in range(2)]
                for e in range(2):
                    mmp = ps_gen.tile([128, CHUNK], F32, name="mmps", tag="gps")
                    for k in range(2):
                        nc.tensor.matmul(
                            mmp[:],
                            wm_sb[k][:, ts(e, 128)],
                            mn_sb[k][:],
                            start=(k == 0),
                            stop=(k == 1),
                        )
                    nc.vector.tensor_scalar_add(msg_sb[e][:], mmp[:], bm_sb[e][:])
                # h1 = W1 @ [x; message]
                for o in range(4):
                    hp = ps_gen.tile([128, CHUNK], F32, name="h1ps", tag="gps")
                    for k in range(4):
                        rhs = x_sb[k][:, ts(j, CHUNK)] if k < 2 else msg_sb[k - 2][:]
                        nc.tensor.matmul(
                            hp[:],
                            w1_sb[k][:, ts(o, 128)],
                            rhs,
                            start=(k == 0),
                            stop=(k == 3),
                        )
                    nc.vector.tensor_copy(h1_sb[o][:, ts(j, CHUNK)], hp[:])
                    nc.vector.bn_stats(stats_sb[o][:, j, :], h1_sb[o][:, ts(j, CHUNK)])

            # ---- phase 3: InstanceNorm + relu + W2 ----
            hn_sb = [data.tile([128, N], BF16, name=f"hn{o}") for o in range(4)]
            for o in range(4):
                mv = small.tile([128, 2], F32, name="mv", tag="mv")
                nc.vector.bn_aggr(mv[:], stats_sb[o][:])
                std = small.tile([128, 1], F32, name="std", tag="std")
                nc.scalar.activation(
                    std[:],
                    mv[:, 1:2],
                    mybir.ActivationFunctionType.Sqrt,
                    bias=eps_sb[:],
                )
                rstd = small.tile([128, 1], F32, name="rstd", tag="rstd")
                nc.vector.reciprocal(rstd[:], std[:])
                nms = small.tile([128, 1], F32, name="nms", tag="nms")
                nc.vector.scalar_tensor_tensor(
                    nms[:],
                    mv[:, 0:1],
                    -1.0,
                    rstd[:],
                    op0=mybir.AluOpType.mult,
                    op1=mybir.AluOpType.mult,
                )
                nc.scalar.activation(
                    hn_sb[o][:],
                    h1_sb[o][:],
                    mybir.ActivationFunctionType.Relu,
                    bias=nms[:],
                    scale=rstd[:],
                )
            for j in range(NCH):
                for c in range(2):
                    op = ps_gen.tile([128, CHUNK], F32, name="ops", tag="gps")
                    for k in range(4):
                        nc.tensor.matmul(
                            op[:],
                            w2_sb[k][:, ts(c, 128)],
                            hn_sb[k][:, ts(j, CHUNK)],
                            start=(k == 0),
                            stop=(k == 3),
                        )
                    ot = msgn.tile([128, CHUNK], F32, name="outt", tag="outt")
                    nc.vector.tensor_scalar_add(ot[:], op[:], b2_sb[c][:])
                    nc.sync.dma_start(
                        out=out_d[ts(c, 128), ts(j, CHUNK)], in_=ot[:]
                    )

    nc.compile()
    return nc


_NC = None


def _get_nc():
    global _NC
    if _NC is None:
        _NC = _build()
    return _NC


def kernel(**inputs):
    x = np.asarray(inputs["x"], np.float32)
    source = np.asarray(inputs["source"], np.float32)
    Wq = np.asarray(inputs["Wq"], np.float32)
    bq = np.asarray(inputs["bq"], np.float32)
    Wk = np.asarray(inputs["Wk"], np.float32)
    bk = np.asarray(inputs["bk"], np.float32)
    Wv = np.asarray(inputs["Wv"], np.float32)
    bv = np.asarray(inputs["bv"], np.float32)
    Wm = np.asarray(inputs["Wm"], np.float32)
    bm = np.asarray(inputs["bm"], np.float32)
    W1 = np.asarray(inputs["W1"], np.float32)
    W2 = np.asarray(inputs["W2"], np.float32)
    b2 = np.asarray(inputs["b2"], np.float32)

    bf = ml_dtypes.bfloat16
    wqT = np.ascontiguousarray(Wq.reshape(H * DH, D).T).astype(bf)
    wkT = np.ascontiguousarray(Wk.reshape(H * DH, D).T).astype(bf)
    wvT = np.ascontiguousarray(Wv.reshape(H * DH, D).T).astype(bf)
    WmP = np.ascontiguousarray(
        Wm.reshape(D, DH, H).transpose(0, 2, 1).reshape(D, D)
    )
    wmT = np.ascontiguousarray(WmP.T).astype(bf)
    w1T = np.ascontiguousarray(W1.T).astype(bf)
    w2T = np.ascontiguousarray(W2.T).astype(bf)
    shared = {
        "wqT": wqT,
        "wkT": wkT,
        "wvT": wvT,
        "wmT": wmT,
        "w1T": w1T,
        "w2T": w2T,
        "bq": np.ascontiguousarray(bq.reshape(D, 1)),
        "bk": np.ascontiguousarray(bk.reshape(D, 1)),
        "bv": np.ascontiguousarray(bv.reshape(1, D)).astype(bf),
        "bm": np.ascontiguousarray(bm.reshape(D, 1)),
        "b2": np.ascontiguousarray(b2.reshape(D, 1)),
    }
    in_maps = []
    for b in range(B):
        m = dict(shared)
        m["x"] = np.ascontiguousarray(x[b]).astype(bf)
        m["src"] = np.ascontiguousarray(source[b]).astype(bf)
        in_maps.append(m)

    nc = _get_nc()
    res = run_bass_kernel_spmd(nc, in_maps, core_ids=list(range(B)))
    return np.stack([res.results[b]["out"] for b in range(B)], axis=0)


# revision 6
# speedup vs baseline: 1.1001x; 1.1001x over previous
"""AttentionalPropagation (SuperGlue-style GNN message passing) on 8 TRN2 NeuronCores.

Sharding: pure data parallel over the batch dim (B=8 -> one batch element per core).
Per-core computation (x, src are (256, 2048) slices; all matmuls in bf16, f32 accum):

  Q = WqS @ x + bq          (256, 2048)   stacked-head layout, c = h*64+dh
  K = WkS @ s + bk          (256, 2048)
  VT = s^T @ WvS^T + bv     (2048, 256)   keys on partitions (transposed layout)
  per head h: S^T[m,n] = K_h[:,m] . Q_h[:,n]  -> exp(S^T/8)  (no max-subtraction;
      scores are O(1) so exp is safe in fp32/bf16)
  msg_u[dh,n] = sum_m exp . VT[m, h*64+dh];  den[n] = sum_m exp   (augmented-V matmul)
  msg = msg_u / den
  h1 = W1x @ x + (W1m@WmP) @ msg       (Wm folded into W1 on host; b1, W1m@bm and
                                        any per-channel bias cancel in InstanceNorm)
  hn = relu(h1 - mean);  out = (W2 * rstd) @ hn + b2   (rstd>0 commutes with relu)

Software-pipelined: chunk j's scores/exps are emitted before chunk j-1's msg/W1
so the Scalar engine (exp; the bottleneck) never starves.
"""

import os
import sys

for _p in ("/opt/trn_rl_repo",):
    if _p not in sys.path:
        sys.path.insert(0, _p)

import numpy as np
import ml_dtypes

import concourse.bass as bass
import concourse.mybir as mybir
from concourse import bacc
from concourse import library_config
from concourse.bass import ts
from concourse.tile import TileContext
from concourse.bass_utils import run_bass_kernel_spmd

F32 = mybir.dt.float32
BF16 = mybir.dt.bfloat16
AF = mybir.ActivationFunctionType
ALU = mybir.AluOpType

B, D, N, M, H, DH = 8, 256, 2048, 2048, 4, 64
EPS = 1e-5
NCH = 4  # n-chunks of 512
CHUNK = 512


def _build():
    nc = bacc.Bacc("TRN2", target_bir_lowering=False, debug=False, num_devices=8)

    x_d = nc.dram_tensor("x", [2, 128, N], BF16, kind="ExternalInput").ap()
    s_d = nc.dram_tensor("src", [2, 128, M], BF16, kind="ExternalInput").ap()
    wq_d = nc.dram_tensor("wqT", [2, 128, D], BF16, kind="ExternalInput").ap()
    wk_d = nc.dram_tensor("wkT", [2, 128, D], BF16, kind="ExternalInput").ap()
    wv_d = nc.dram_tensor("wvT", [2, 128, D], BF16, kind="ExternalInput").ap()
    w1_d = nc.dram_tensor("w1T", [4, 128, 2 * D], BF16, kind="ExternalInput").ap()
    w2_d = nc.dram_tensor("w2T", [4, 128, D], BF16, kind="ExternalInput").ap()
    # biases packed as columns: [bq, bk, b2] per channel tile
    bias_d = nc.dram_tensor("bias", [2, 128, 3], F32, kind="ExternalInput").ap()
    bv_d = nc.dram_tensor("bv", [1, D], BF16, kind="ExternalInput").ap()
    out_d = nc.dram_tensor("out", [D, N], F32, kind="ExternalOutput").ap()

    with TileContext(nc) as tc:
        nc.gpsimd.load_library(library_config.attn)
        with (
            tc.tile_pool(name="const", bufs=1) as const,
            tc.tile_pool(name="data", bufs=1) as data,
            tc.tile_pool(name="reuse", bufs=2) as reuse,
            tc.tile_pool(name="exps", bufs=6) as exps,
            tc.tile_pool(name="small", bufs=2) as small,
            tc.tile_pool(name="msgn", bufs=4) as msgn,
            tc.tile_pool(name="ps_sc", bufs=2, space="PSUM") as ps_sc,
            tc.tile_pool(name="ps_msg", bufs=2, space="PSUM") as ps_msg,
            tc.tile_pool(name="ps_gen", bufs=2, space="PSUM") as ps_gen,
        ):
            # ---- inputs + weights (few large DMAs; x/wq first for fast start) ----
            x_sb = data.tile([128, 2, N], BF16, name="x")
            wq_sb = const.tile([128, 2, D], BF16, name="wq")
            nc.sync.dma_start(out=x_sb[:], in_=x_d.rearrange("k p n -> p k n"))
            nc.sync.dma_start(out=wq_sb[:], in_=wq_d.rearrange("k p n -> p k n"))
            s_sb = reuse.tile([128, 2, M], BF16, name="s", tag="big")
            wk_sb = const.tile([128, 2, D], BF16, name="wk")
            wv_sb = const.tile([128, 2, D], BF16, name="wv")
            nc.sync.dma_start(out=s_sb[:], in_=s_d.rearrange("k p n -> p k n"))
            nc.sync.dma_start(out=wk_sb[:], in_=wk_d.rearrange("k p n -> p k n"))
            nc.sync.dma_start(out=wv_sb[:], in_=wv_d.rearrange("k p n -> p k n"))
            bias_sb = const.tile([128, 2, 3], F32, name="bias")
            nc.sync.dma_start(out=bias_sb[:], in_=bias_d.rearrange("k p n -> p k n"))
            bv_bc = const.tile([128, D], BF16, name="bvbc")
            bv_src = bass.AP(
                tensor=bv_d.tensor, offset=bv_d.offset, ap=[[0, 128]] + bv_d.ap[1:]
            )
            nc.sync.dma_start(out=bv_bc[:], in_=bv_src)
            w1_sb = const.tile([128, 4, 2 * D], BF16, name="w1")
            nc.sync.dma_start(out=w1_sb[:], in_=w1_d.rearrange("k p n -> p k n"))
            w2_sb = const.tile([128, 4, D], BF16, name="w2")
            nc.sync.dma_start(out=w2_sb[:], in_=w2_d.rearrange("k p n -> p k n"))
            eps_sb = const.tile([128, 1], F32, name="eps")
            nc.vector.memset(eps_sb[:], EPS)

            # ---- phase 1: QKV projections (weight-stationary: 1 LDW per 4 MMs) ----
            q_sb = data.tile([128, 2, N], BF16, name="q")
            k_sb = data.tile([128, 2, M], BF16, name="k")
            for dst, w_sb, src_t, b_col in (
                (q_sb, wq_sb, x_sb, 0),
                (k_sb, wk_sb, s_sb, 1),
            ):
                for c in range(2):  # output channel tile
                    ps = [ps_sc.tile([128, 2, CHUNK], F32, name="qk", tag="scps")
                          for _ in range(2)]
                    for k in range(2):
                        for j in range(NCH):
                            nc.tensor.matmul(
                                ps[j // 2][:, j % 2, :],
                                w_sb[:, k, ts(c, 128)],
                                src_t[:, k, ts(j, CHUNK)],
                                start=(k == 0),
                                stop=(k == 1),
                            )
                    for half in range(2):
                        nc.vector.tensor_scalar_add(
                            dst[:, c, ts(half, 2 * CHUNK)],
                            ps[half][:],
                            bias_sb[:, c, b_col : b_col + 1],
                        )
            # V^T: (m, c) layout, 65-wide per-head blocks with a ones column
            vT_sb = [data.tile([128, H, DH + 1], BF16, name=f"vT{t}") for t in range(16)]
            for t in range(16):
                vp = ps_gen.tile([128, D], F32, name="vps", tag="gps")
                for k in range(2):
                    nc.tensor.matmul(
                        vp[:],
                        s_sb[:, k, ts(t, 128)],
                        wv_sb[:, k, :],
                        start=(k == 0),
                        stop=(k == 1),
                    )
                nc.vector.tensor_add(
                    vT_sb[t][:, :, 0:DH],
                    vp[:].rearrange("p (h d) -> p h d", h=H),
                    bv_bc[:].rearrange("p (h d) -> p h d", h=H),
                )
                nc.vector.memset(vT_sb[t][:, :, DH : DH + 1], 1.0)

            # ---- phase 2: attention, software-pipelined msg/W1 one chunk behind ----
            h1_sb = data.tile([128, 4, N], BF16, name="h1")
            stats_sb = data.tile([128, 4, NCH, 6], F32, name="stats")
            eS = {}   # (j, h) -> expS tile (128, 16, CHUNK)
            mn = {}   # (j, p) -> normalized msg pair tile (128, CHUNK)

            def emit_scores_exps(j):
                for p in range(2):  # head pair
                    for h2 in range(2):
                        eS[(j, 2 * p + h2)] = exps.tile(
                            [128, 16, CHUNK], BF16, name="expS", tag="expS"
                        )
                    for s in range(8):  # super-tile: 2 m-tiles
                        scp = [
                            ps_sc.tile([128, 2, CHUNK], F32, name="sc", tag="scps")
                            for _ in range(2)
                        ]
                        for jj in range(2):
                            mt = 2 * s + jj
                            for h2 in range(2):
                                nc.tensor.matmul(
                                    scp[h2][:, jj, :],
                                    k_sb[ts(h2, DH), p, ts(mt, 128)],
                                    q_sb[ts(h2, DH), p, ts(j, CHUNK)],
                                    start=True,
                                    stop=True,
                                )
                        for h2 in range(2):
                            nc.scalar.activation(
                                eS[(j, 2 * p + h2)][:, 2 * s : 2 * s + 2, :],
                                scp[h2][:],
                                AF.Exp,
                                scale=1.0 / 8.0,
                            )

            def emit_msg_h1(j):
                # msg per head (augmented-V: psum row 64 = denominator)
                for p in range(2):
                    mn[(j, p)] = msgn.tile([128, CHUNK], BF16, name="mn", tag="mn")
                for h in range(4):
                    p, h2 = h // 2, h % 2
                    mp = ps_msg.tile([DH + 1, CHUNK], F32, name="msgps")
                    for mt in range(16):
                        nc.tensor.matmul(
                            mp[:],
                            vT_sb[mt][:, h, :],
                            eS[(j, h)][:, mt, :],
                            start=(mt == 0),
                            stop=(mt == 15),
                        )
                    del eS[(j, h)]
                    den = small.tile([1, CHUNK], F32, name="den", tag="den")
                    nc.vector.tensor_copy(den[:], mp[DH : DH + 1, :])
                    rden = small.tile([1, CHUNK], F32, name="rden", tag="rden")
                    nc.vector.reciprocal_approx_fast(rden[:], den[:])
                    rbc = small.tile([DH, CHUNK], F32, name="rbc", tag="rbc")
                    nc.gpsimd.partition_broadcast(rbc[:], rden[:])
                    nc.vector.tensor_mul(mn[(j, p)][ts(h2, DH), :], mp[0:DH, :], rbc[:])
                # h1 = W1x @ x + W1mWm @ msg
                for o in range(4):
                    hp = ps_gen.tile([128, CHUNK], F32, name="h1ps", tag="gps")
                    for k in range(4):
                        rhs = (
                            x_sb[:, k, ts(j, CHUNK)] if k < 2 else mn[(j, k - 2)][:]
                        )
                        nc.tensor.matmul(
                            hp[:],
                            w1_sb[:, k, ts(o, 128)],
                            rhs,
                            start=(k == 0),
                            stop=(k == 3),
                        )
                    nc.vector.tensor_copy(h1_sb[:, o, ts(j, CHUNK)], hp[:])
                    nc.vector.bn_stats(
                        stats_sb[:, o, j, :], h1_sb[:, o, ts(j, CHUNK)]
                    )

            for j in range(NCH):
                emit_scores_exps(j)
                if j > 0:
                    emit_msg_h1(j - 1)
            emit_msg_h1(NCH - 1)

            # ---- phase 3: InstanceNorm (relu on DVE, rstd folded into W2) ----
            hn_sb = reuse.tile([128, 4, N], BF16, name="hn", tag="big")
            mean = small.tile([128, 4], F32, name="mean", tag="mean")
            for o in range(4):
                mv = small.tile([128, 2], F32, name="mv", tag="mv")
                nc.vector.bn_aggr(mv[:], stats_sb[:, o, :, :])
                nc.vector.tensor_copy(mean[:, o : o + 1], mv[:, 0:1])
                std = small.tile([128, 1], F32, name="std", tag="std")
                nc.scalar.activation(std[:], mv[:, 1:2], AF.Sqrt, bias=eps_sb[:])
                rstd = small.tile([128, 1], F32, name="rstd", tag="rstd")
                nc.vector.reciprocal(rstd[:], std[:])
                nc.vector.tensor_scalar_mul(w2_sb[:, o, :], w2_sb[:, o, :], rstd[:])
            for j in range(NCH):
                for o in range(4):
                    nc.vector.tensor_scalar(
                        hn_sb[:, o, ts(j, CHUNK)],
                        h1_sb[:, o, ts(j, CHUNK)],
                        mean[:, o : o + 1],
                        0.0,
                        op0=ALU.subtract,
                        op1=ALU.max,
                    )
                for c in range(2):
                    op = ps_gen.tile([128, CHUNK], F32, name="ops", tag="gps")
                    for k in range(4):
                        nc.tensor.matmul(
                            op[:],
                            w2_sb[:, k, ts(c, 128)],
                            hn_sb[:, k, ts(j, CHUNK)],
                            start=(k == 0),
                            stop=(k == 3),
                        )
                    ot = small.tile([128, CHUNK], F32, name="outt", tag="outt")
                    nc.vector.tensor_scalar_add(
                        ot[:], op[:], bias_sb[:, c, 2:3]
                    )
                    nc.sync.dma_start(out=out_d[ts(c, 128), ts(j, CHUNK)], in_=ot[:])

    nc.compile()
    return nc


_NC = None


def _get_nc():
    global _NC
    if _NC is None:
        _NC = _build()
    return _NC


def kernel(**inputs):
    x = np.asarray(inputs["x"], np.float32)
    source = np.asarray(inputs["source"], np.float32)
    Wq = np.asarray(inputs["Wq"], np.float32)
    bq = np.asarray(inputs["bq"], np.float32)
    Wk = np.asarray(inputs["Wk"], np.float32)
    bk = np.asarray(inputs["bk"], np.float32)
    Wv = np.asarray(inputs["Wv"], np.float32)
    bv = np.asarray(inputs["bv"], np.float32)
    Wm = np.asarray(inputs["Wm"], np.float64)
    bm = np.asarray(inputs["bm"], np.float64)
    W1 = np.asarray(inputs["W1"], np.float64)
    W2 = np.asarray(inputs["W2"], np.float32)
    b2 = np.asarray(inputs["b2"], np.float32)

    bf = ml_dtypes.bfloat16
    wqT = np.ascontiguousarray(Wq.reshape(H * DH, D).T).astype(bf).reshape(2, 128, D)
    wkT = np.ascontiguousarray(Wk.reshape(H * DH, D).T).astype(bf).reshape(2, 128, D)
    wvT = np.ascontiguousarray(Wv.reshape(H * DH, D).T).astype(bf).reshape(2, 128, D)
    # message-channel permutation (dh-major -> head-major) folded into Wm
    WmP = Wm.reshape(D, DH, H).transpose(0, 2, 1).reshape(D, D)
    # fold Wm into W1's message half; b1 and W1m@bm cancel in InstanceNorm
    W1mWm = W1[:, D:] @ WmP
    w1T = (
        np.vstack([W1[:, :D].T, W1mWm.T])
        .astype(np.float32)
        .astype(bf)
        .reshape(4, 128, 2 * D)
    )
    w2T = np.ascontiguousarray(W2.T).astype(bf).reshape(4, 128, D)
    bias = np.stack(
        [bq.reshape(D).astype(np.float32), bk.reshape(D).astype(np.float32),
         b2.reshape(D)], axis=1
    ).reshape(2, 128, 3)
    shared = {
        "wqT": wqT,
        "wkT": wkT,
        "wvT": wvT,
        "w1T": np.ascontiguousarray(w1T),
        "w2T": w2T,
        "bias": np.ascontiguousarray(bias),
        "bv": np.ascontiguousarray(bv.reshape(1, D)).astype(bf),
    }
    in_maps = []
    for b in range(B):
        m = dict(shared)
        m["x"] = np.ascontiguousarray(x[b]).astype(bf).reshape(2, 128, N)
        m["src"] = np.ascontiguousarray(source[b]).astype(bf).reshape(2, 128, M)
        in_maps.append(m)

    nc = _get_nc()
    res = run_bass_kernel_spmd(nc, in_maps, core_ids=list(range(B)))
    return np.stack([res.results[b]["out"] for b in range(B)], axis=0)
